# revision 1
# baseline (speedup 1.0000x reference)
"""Trainium2 Bass kernel for LpAlignEntropyLoss (B=2048, D=128, 2 views).

loss = mean_i ||z0_i - z1_i + eps||  -  0.5 * sum_v mean_i [ logsumexp_{j!=i}(-||zv_i - zv_j + eps||) - log(B-1) ]

Strategy (8 NeuronCores, batch-row sharded, 256 rows/core):
  dist^2[i,j] = n_i + n_j - 2 * z_i . z_j   (matmul trick, bf16 TensorE)
  - Each core gets z^T column-ROTATED so its own 256 rows are columns 0..255:
    the distance-matrix diagonal then sits at a compile-time-known position.
  - The diagonal is self-masked by accumulating -BIG*I into PSUM via a tiny
    identity matmul => exp(-sqrt(...)) underflows to exactly 0.
  - ScalarE pass 1: dist = Sqrt(-2*psum + n_row)   (bias = per-partition n_i)
  - ScalarE pass 2: Exp(-dist) with fused accum_out row-sum.
  - Align term: DVE diff+square of the first 256 columns, ones-matmul to
    reduce over D (partition axis).
  Host finishes the O(B) tail: log of the row-sums, sqrt of align rows, means.

eps=1e-8 is below fp32 ulp of every operand magnitude here; dropping it is
exact at fp32 resolution.
"""
import numpy as np
import ml_dtypes
from contextlib import ExitStack

B = 2048
D = 128
N_CORES = 8
R = B // N_CORES          # 256 rows per core
NCH = R // 128            # 2 row-chunks of 128 partitions
BIG = float(2 ** 20)
TAU = 1.0
LOG_NM1 = float(np.log(B - 1))

_cache: dict = {}


def _build():
    import concourse.tile as tile
    from concourse import bacc, mybir

    f32 = mybir.dt.float32
    bf16 = mybir.dt.bfloat16
    AF = mybir.ActivationFunctionType

    nc = bacc.Bacc("TRN2", target_bir_lowering=False, debug=False,
                   num_devices=N_CORES)

    zt_d = [nc.dram_tensor(f"zt{v}", [D, B], bf16, kind="ExternalInput").ap()
            for v in (0, 1)]
    nh_d = [nc.dram_tensor(f"nh{v}", [1, B], bf16, kind="ExternalInput").ap()
            for v in (0, 1)]
    nrow_d = nc.dram_tensor("nrow", [128, 2 * NCH], f32, kind="ExternalInput").ap()
    ident_d = nc.dram_tensor("ident", [128, 128], bf16, kind="ExternalInput").ap()
    ibig_d = nc.dram_tensor("ibig", [128, 128], bf16, kind="ExternalInput").ap()
    rowsums_d = nc.dram_tensor("rowsums", [2 * NCH, 128], f32,
                               kind="ExternalOutput").ap()
    alignsq_d = nc.dram_tensor("alignsq", [1, R], f32, kind="ExternalOutput").ap()

    with tile.TileContext(nc) as tc, ExitStack() as ctx:
        consts = ctx.enter_context(tc.tile_pool(name="consts", bufs=1))
        ztp = ctx.enter_context(tc.tile_pool(name="ztp", bufs=1))
        psum = ctx.enter_context(tc.tile_pool(name="psum", bufs=2, space="PSUM"))
        distp = ctx.enter_context(tc.tile_pool(name="distp", bufs=4))
        dumpp = ctx.enter_context(tc.tile_pool(name="dumpp", bufs=2))
        accp = ctx.enter_context(tc.tile_pool(name="accp", bufs=4))
        alnp = ctx.enter_context(tc.tile_pool(name="alnp", bufs=1))

        sb_zt = []
        for v in (0, 1):
            t_ = ztp.tile([D, B], bf16, tag=f"zt{v}")
            nc.sync.dma_start(t_[:], zt_d[v])
            sb_zt.append(t_)
        sb_nh = []
        for v in (0, 1):
            t_ = consts.tile([1, B], bf16, tag=f"nh{v}")
            nc.sync.dma_start(t_[:], nh_d[v])
            sb_nh.append(t_)
        sb_nrow = consts.tile([128, 2 * NCH], f32, tag="nrow")
        nc.sync.dma_start(sb_nrow[:], nrow_d)
        sb_ident = consts.tile([128, 128], bf16, tag="ident")
        nc.sync.dma_start(sb_ident[:], ident_d)
        sb_ibig = consts.tile([128, 128], bf16, tag="ibig")
        nc.sync.dma_start(sb_ibig[:], ibig_d)
        ones = consts.tile([128, 128], bf16, tag="ones")
        nc.vector.memset(ones[:], 1.0)

        # Phase A: matmuls + Sqrt passes (one ACT table set)
        dists = {}
        for v in (0, 1):
            for t in range(NCH):
                P = psum.tile([128, B], f32, tag="P")
                lhsT = sb_zt[v][:, t * 128:(t + 1) * 128]
                for s in range(4):
                    sl = slice(s * 512, (s + 1) * 512)
                    nc.tensor.matmul(P[:, sl], lhsT, sb_zt[v][:, sl],
                                     start=True, stop=False)
                    nc.tensor.matmul(P[:, sl], ones[0:1, :], sb_nh[v][0:1, sl],
                                     start=False, stop=(s != 0))
                dg = slice(t * 128, (t + 1) * 128)
                nc.tensor.matmul(P[:, dg], sb_ident[:], sb_ibig[:],
                                 start=False, stop=True)
                dist = distp.tile([128, B], f32, tag="dist")
                idx = v * NCH + t
                nc.scalar.activation(dist[:], P[:], AF.Sqrt,
                                     bias=sb_nrow[:, idx:idx + 1], scale=-2.0)
                dists[(v, t)] = dist

        # Phase B: Exp passes with fused row-sum (second ACT table set)
        for v in (0, 1):
            for t in range(NCH):
                dmp = dumpp.tile([128, B], bf16, tag="dump")
                acc = accp.tile([128, 1], f32, tag="acc")
                nc.scalar.activation(dmp[:], dists[(v, t)][:], AF.Exp,
                                     scale=-1.0 / TAU, accum_out=acc[:])
                idx = v * NCH + t
                nc.sync.dma_start(rowsums_d[idx:idx + 1, :], acc[:])

        # Align term: ||z0_i - z1_i||^2 for this core's 256 rows
        adiff = alnp.tile([128, R], bf16, tag="adiff")
        nc.vector.tensor_sub(adiff[:], sb_zt[0][:, :R], sb_zt[1][:, :R])
        asq = alnp.tile([128, R], bf16, tag="asq")
        nc.vector.tensor_mul(asq[:], adiff[:], adiff[:])
        aps = psum.tile([1, R], f32, tag="P")
        nc.tensor.matmul(aps[:], ones[:, 0:1], asq[:], start=True, stop=True)
        asb = alnp.tile([1, R], f32, tag="asb")
        nc.vector.tensor_copy(asb[:], aps[:])
        nc.sync.dma_start(alignsq_d[0:1, :], asb[:])

    nc.compile()
    return nc


def _prep_inputs(z0: np.ndarray, z1: np.ndarray):
    """Per-core input maps: rotate columns so core c's rows come first."""
    bf = ml_dtypes.bfloat16
    zs = [np.ascontiguousarray(z0, np.float32), np.ascontiguousarray(z1, np.float32)]
    norms = [(z.astype(np.float64) ** 2).sum(-1) for z in zs]  # [B] exact-ish
    eye = np.eye(128, dtype=np.float32)
    ident = eye.astype(bf)
    ibig = (-BIG * eye).astype(bf)
    in_maps = []
    for c in range(N_CORES):
        order = (np.arange(B) + c * R) % B
        m = {"ident": ident, "ibig": ibig}
        nrow = np.empty((128, 2 * NCH), np.float32)
        for v in (0, 1):
            zr = zs[v][order]                       # [B, D] rotated
            m[f"zt{v}"] = np.ascontiguousarray(zr.T).astype(bf)   # [D, B]
            m[f"nh{v}"] = (-0.5 * norms[v][order]).astype(np.float32)\
                .astype(bf).reshape(1, B)
            for t in range(NCH):
                nrow[:, v * NCH + t] = norms[v][order[t * 128:(t + 1) * 128]]\
                    .astype(np.float32)
        m["nrow"] = nrow
        in_maps.append(m)
    return in_maps


def kernel(z0: np.ndarray, z1: np.ndarray) -> np.ndarray:
    from concourse.bass_utils import run_bass_kernel_spmd

    if "nc" not in _cache:
        _cache["nc"] = _build()
    nc = _cache["nc"]

    in_maps = _prep_inputs(z0, z1)
    res = run_bass_kernel_spmd(nc, in_maps, core_ids=list(range(N_CORES)))

    rowsums = np.empty((2, B), np.float64)   # [view, global row]
    alignsq = np.empty((B,), np.float64)
    for c in range(N_CORES):
        out = res.results[c]
        rs = out["rowsums"].astype(np.float64)      # [2*NCH, 128]
        for v in (0, 1):
            for t in range(NCH):
                g0 = c * R + t * 128
                rowsums[v, g0:g0 + 128] = rs[v * NCH + t]
        alignsq[c * R:(c + 1) * R] = out["alignsq"][0].astype(np.float64)

    align_loss = np.sqrt(alignsq).mean()
    lme = np.log(rowsums) - LOG_NM1             # [2, B]
    entropy_loss = lme.mean()
    return np.float32(align_loss - entropy_loss)

